# revision 52
# baseline (speedup 1.0000x reference)
"""Hawk (RG-LRU recurrent block) Trainium2 kernel, 8-core SPMD.

Sharding: data-parallel over B (2 groups of 4 cores) x sequence-parallel over
T (4 chunks of 1024 tokens per batch element). The diagonal linear recurrence
h_t = a_t * h_{t-1} + b_t is computed chunk-locally with the hardware
tensor_tensor_scan instruction (fp32 state feedback), then stitched across
cores with one small AllGather of per-chunk scan summaries (A = prod a,
b = local final h) and the per-core correction h = h_local + cumprod(a)*carry.

Layout on device: hidden channels on partitions, time on the free dimension.

Optimizations over the f32r baseline (412us -> ~214us in the cost model):
  - input and output projections run as fp8 e4m3 DoubleRow matmuls (2 K-blocks
    per instruction at 0.5 cycles/row = 4x f32r throughput) using a hi+lo
    split: W@x ~= Wh@xh + Wl@xh + Wh@xl with both operands split host-side
    (weights scaled by 32 into fp8 range; descale folded into the per-
    partition ACT scale at the PSUM read). Residual-of-residual error is
    ~0.1%, BETTER than plain bf16.
  - the rec-gate matmul is a single (uncorrected) fp8 DoubleRow pass: alpha
    error tolerance scales with the per-channel forget rate, which is tiny
    exactly for the long-memory channels. The input-gate matmul stays bf16
    (its split-3 variant shifts the bottleneck to the vector engine).
  - sigmoid via tanh identity keeps phase 2 in one ACT table set (tanh+exp);
    sqrt batches amortize the only other table load. ~22 table loads -> ~10.
  - weights are stored pre-transposed/pre-quantized in DRAM so every load is
    a contiguous [128, N] DMA (~128 descriptors); streams are spread across
    the SP and ACT HWDGE queues with explicit prefetch depth so no in-order
    queue blocks a later phase's loads. The baseline serialized ~400us of
    descriptor generation on the SP sequencer.
  - conv runs on DVE with the unshifted tap folded into the PSUM->SBUF copy
    (ACT Identity with per-partition scale+bias); bf16 operands throughout.
  - P = cumprod(alpha) stays in SBUF (bf16, single rounding per element);
    gelu(gate) is folded into h_loc and P in place during the collective so
    the post-carry critical path is one fused STT per channel block.
  - phase 2.5 (gate projection) fully hides the 15us-fixed-cost collective;
    the carry composition and gh production overlap its tail.
"""
import contextlib
import os

os.environ.setdefault("JAX_COMPILATION_CACHE_DIR", "/tmp/jax_cache_hawk")

import numpy as np

import concourse.bacc as bacc
import concourse.mybir as mybir
import concourse.tile as tile
from concourse.bass_utils import run_bass_kernel_spmd

F32 = mybir.dt.float32
BF16 = mybir.dt.bfloat16
FP8 = mybir.dt.float8e4
AF = mybir.ActivationFunctionType
OP = mybir.AluOpType
DR = mybir.MatmulPerfMode.DoubleRow

NPBF16 = mybir.dt.np(BF16)
NPFP8 = mybir.dt.np(FP8)

DIM = 1024
HID = 1536
KCONV = 4
B = 2
T = 4096
C_CONST = 8.0

NCORE = 8
TC = T // 4          # tokens per core
NH = HID // 128      # 12 hidden chunks
NDC = DIM // 128     # 8 dim chunks
NP2 = NH // 2        # 6 hidden chunk pairs (fp8 DoubleRow)
TPAD = TC + 4        # 1028: col0 zero, cols1:4 = 3-token left overlap
NCC = 11             # const columns per hidden chunk
WRG_SCALE = 64.0     # fp8 scale for W_gate

_CACHE: dict = {}


def _build(debug=False):
    nc = bacc.Bacc("TRN2", target_bir_lowering=False, debug=False,
                   num_devices=NCORE, dynamic_dma_scratch_size=8192)

    xt8h = nc.dram_tensor("xt8h", [4, 128, 4, 2, 256], FP8,
                          kind="ExternalInput").ap()
    xt8l = nc.dram_tensor("xt8l", [4, 128, 4, 2, 256], FP8,
                          kind="ExternalInput").ap()
    xpre = nc.dram_tensor("xpre", [128, 4, 2, 4], FP8,
                          kind="ExternalInput").ap()
    wp8h = nc.dram_tensor("wp8h", [24, 128, 4, 2, 128], FP8,
                          kind="ExternalInput").ap()
    wp8l = nc.dram_tensor("wp8l", [24, 128, 4, 2, 128], FP8,
                          kind="ExternalInput").ap()
    wig = nc.dram_tensor("wig", [NH, 128, NH * 128], BF16,
                         kind="ExternalInput").ap()
    wrg = nc.dram_tensor("wrg", [NH, 128, NP2, 2, 128], FP8,
                         kind="ExternalInput").ap()
    wout = nc.dram_tensor("wout", [NDC, 128, NH * 128], BF16,
                          kind="ExternalInput").ap()
    consts = nc.dram_tensor("consts", [128, NCC * NH + 4], F32,
                            kind="ExternalInput").ap()
    out = nc.dram_tensor("out", [NDC, 128, TC], BF16, kind="ExternalOutput").ap()
    if debug:
        dbg_ucb = nc.dram_tensor("dbg_ucb", [NH, 128, TC], F32,
                                 kind="ExternalOutput").ap()
        dbg_g0 = nc.dram_tensor("dbg_g0", [6, 128, TC], F32,
                                kind="ExternalOutput").ap()
        dbg_sc = nc.dram_tensor("dbg_sc", [4, 128, TC], F32,
                                kind="ExternalOutput").ap()
        dbg_sm = nc.dram_tensor("dbg_sm", [3, 128, 24], F32,
                                kind="ExternalOutput").ap()

    with tile.TileContext(nc) as tc:
        with contextlib.ExitStack() as stk:
            def pool(name, bufs, space="SBUF"):
                return stk.enter_context(
                    tc.tile_pool(name=name, bufs=bufs, space=space))

            xtp = pool("xtp", NDC)
            ucg = pool("ucg", NH)          # ucb then gh
            u8p = pool("u8p", NP2)
            hlp = pool("hlp", NH)
            ptp = pool("ptp", NH)
            t16 = pool("t16", 6)
            b16p = pool("b16p", 6)         # upre/tmp (conv)
            alp = pool("alp", 5)
            a2p = pool("a2p", 5)
            bhp = pool("bhp", 6)
            xbp = pool("xbp", 2)
            wp = pool("wp", 5)
            wrgp = pool("wrgp", 3)
            wigp = pool("wigp", 3)
            wop = pool("wop", 3)
            cst = pool("cst", 1)
            ps = pool("ps", 3, space="PSUM")
            psb = pool("psb", 1, space="PSUM")
            dram = pool("dram", 1, space="DRAM")

            # ---- x^T + first weights (PE work is gated on these) ----
            # fp8 hi+lo split, DoubleRow-paired over input-channel blocks
            wpr_t: dict = {}

            def fetch_proj1(mc):
                if mc >= NH or mc in wpr_t:
                    return
                wh = wp.tile([128, 4, 2, 128], FP8, tag="wh", name="wh")
                nc.sync.dma_start(wh[:], wp8h[mc])
                wl = wp.tile([128, 4, 2, 128], FP8, tag="wl", name="wl")
                nc.sync.dma_start(wl[:], wp8l[mc])
                wpr_t[mc] = (wh, wl)

            wh0 = wp.tile([128, 4, 2, 128], FP8, tag="wh", name="wh")
            nc.sync.dma_start(wh0[:], wp8h[0])
            x8h_t = []
            x8l_t = []
            for p in range(4):
                xh = xtp.tile([128, 4, 2, 256], FP8, tag="xt", name="xh")
                nc.sync.dma_start(xh[:], xt8h[p])
                x8h_t.append(xh)
            wl0 = wp.tile([128, 4, 2, 128], FP8, tag="wl", name="wl")
            nc.sync.dma_start(wl0[:], wp8l[0])
            wpr_t[0] = (wh0, wl0)
            for p in range(4):
                xl = xtp.tile([128, 4, 2, 256], FP8, tag="xt", name="xl")
                nc.sync.dma_start(xl[:], xt8l[p])
                x8l_t.append(xl)
            fetch_proj1(1)

            # ---- constants: one DMA (ACT queue; conv reads them there) ----
            ct = cst.tile([128, NCC * NH + 4], F32, tag="ct", name="ct")
            nc.scalar.dma_start(ct[:], consts[:])

            def cc_col(gc, c):
                return ct[:, gc * NCC + c: gc * NCC + c + 1]

            S_loc = cst.tile([128, 24], F32, tag="sloc", name="S_loc")
            p2_t = cst.tile([128, NH], F32, tag="p2", name="p2_t")
            p3_t = cst.tile([128, NH], F32, tag="p3", name="p3_t")
            c_t = cst.tile([128, NH], F32, tag="cc", name="c_t")

            def proj_matmuls(pt, p3t, wh, wl):
                # W@x = Wh@xh + Wh@xl + Wl@xh (lo*lo dropped), all fp8
                # DoubleRow at 4x f32r rate; scale 32 on W folded out at the
                # PSUM read.
                passes = ((wh, x8h_t), (wl, x8h_t), (wh, x8l_t))
                for q in range(4):
                    for n, (w_, x_) in enumerate(passes):
                        for p in range(4):
                            nc.tensor.matmul(
                                pt[:, q * 256:(q + 1) * 256],
                                w_[:, p],
                                x_[p][:, q],
                                start=(n == 0 and p == 0),
                                stop=(n == 2 and p == 3),
                                perf_mode=DR)
                if p3t is not None:
                    for p in range(4):
                        nc.tensor.matmul(
                            p3t[:, 0:4],
                            wh[:, p],
                            xpre_t[:, p],
                            start=(p == 0), stop=(p == 3),
                            perf_mode=DR)

            # ---- phase 1: u half of proj + causal conv ----
            ucb_t = []
            u8_t = []
            for p in range(NP2):
                u8 = u8p.tile([128, 4, 2, 256], FP8, tag="u8", name="u8")
                u8_t.append(u8)
            xpre_t = cst.tile([128, 4, 2, 4], FP8, tag="xpre", name="xpre_t")
            nc.sync.dma_start(xpre_t[:], xpre[:])
            for mc in range(NH):
                fetch_proj1(mc)
                wh, wl = wpr_t[mc]
                pt = ps.tile([128, TC], F32, tag="ps", name="pt")
                p3t = psb.tile([128, TC], F32, tag="psb", name="p3t")
                proj_matmuls(pt, p3t, wh, wl)
                fetch_proj1(mc + 2)
                upre = b16p.tile([128, TPAD], BF16, tag="b16", name="upre")
                tmp = b16p.tile([128, TPAD], BF16, tag="b16", name="tmp")
                ucb = ucg.tile([128, TC], BF16, tag="ucg", name="ucb")
                # u_c[t] = sum_k w_k * u_pre[t+k+1] + conv_b; the k=3 tap is
                # unshifted, so it folds into the PSUM->SBUF read via ACT
                # scale/bias (tmp = w_3 * u_pre_body + conv_b); the last two
                # blocks run at half-tile granularity so the phase-2 gate
                # matmuls are not gated on a full-width conv chain.
                halves = (0, 1) if mc >= NH - 2 else (None,)
                for hh in halves:
                    if hh is None:
                        ps_sl = slice(0, TC)
                    else:
                        ps_sl = slice(hh * 512, (hh + 1) * 512)
                    up_sl = slice(4 + ps_sl.start, 4 + ps_sl.stop)
                    nc.scalar.activation(upre[:, up_sl], pt[:, ps_sl],
                                         AF.Identity, scale=1.0 / 32.0)
                    nc.scalar.activation(tmp[:, ps_sl], pt[:, ps_sl],
                                         AF.Identity, bias=cc_col(mc, 4),
                                         scale=cc_col(mc, 10))
                nc.vector.tensor_scalar(upre[:, 0:4], p3t[:, 0:4],
                                        1.0 / 32.0, None, OP.mult)
                for hh in halves:
                    t_sl = (slice(0, TC) if hh is None
                            else slice(hh * 512, (hh + 1) * 512))
                    n_ = t_sl.stop - t_sl.start
                    for k in (0, 1):
                        nc.vector.scalar_tensor_tensor(
                            tmp[:, t_sl],
                            upre[:, 1 + k + t_sl.start:1 + k + t_sl.start + n_],
                            cc_col(mc, k), tmp[:, t_sl], OP.mult, OP.add)
                    nc.vector.scalar_tensor_tensor(
                        ucb[:, t_sl],
                        upre[:, 3 + t_sl.start:3 + t_sl.start + n_],
                        cc_col(mc, 2), tmp[:, t_sl], OP.mult, OP.add)
                ucb_t.append(ucb)
                # fp8 copy for the rec-gate DoubleRow matmul, th-interleaved
                p, i = divmod(mc, 2)
                nc.scalar.copy(
                    u8_t[p][:, :, i, :],
                    ucb[:].rearrange("k (q c) -> k q c", q=4))


            if debug:
                for mc in range(NH):
                    dt_ = alp.tile([128, TC], F32, tag="al", name="dt_")
                    nc.vector.tensor_copy(dt_[:, 0:TC], ucb_t[mc][:])
                    nc.sync.dma_start(dbg_ucb[mc], dt_[:, 0:TC])

            # ---- phase 2: gates + scans, in ACT-table batches ----
            # tanh/exp share one ACT table set; Sqrt lives in another, so
            # the per-batch Sqrt ops are grouped to amortize table loads.
            hl_t: list = [None] * NH
            pt_t: list = [None] * NH

            wi_t: dict = {}
            wr_t: dict = {}

            def fetch_gates(gc):
                if gc >= NH:
                    return
                wi = wigp.tile([128, NH * 128], BF16, tag="wi", name="wi")
                nc.sync.dma_start(wi[:], wig[gc])
                wi_t[gc] = wi
                wr = wrgp.tile([128, NP2, 2, 128], FP8, tag="wr", name="wr")
                nc.sync.dma_start(wr[:], wrg[gc])
                wr_t[gc] = wr

            fetch_gates(0)
            fetch_gates(1)

            def emit_2a(gcs, A2_t, AL_t, SI_t):
                for gc in gcs:
                    # input gate: bf16 (needs only ucb -> starts earliest)
                    pig = ps.tile([128, TC], F32, tag="ps", name="pig")
                    for th in range(2):
                        for hc in range(NH):
                            nc.tensor.matmul(
                                pig[:, th * 512:(th + 1) * 512],
                                wi_t[gc][:, hc * 128:(hc + 1) * 128],
                                ucb_t[hc][:, th * 512:(th + 1) * 512],
                                start=(hc == 0), stop=(hc == NH - 1))
                    # rec gate: fp8 e4m3 DoubleRow (2 K-chunks per matmul)
                    prg = psb.tile([128, TC], F32, tag="psb", name="prg")
                    for q in range(4):
                        for p in range(NP2):
                            nc.tensor.matmul(
                                prg[:, q * 256:(q + 1) * 256],
                                wr_t[gc][:, p],
                                u8_t[p][:, q],
                                start=(p == 0), stop=(p == NP2 - 1),
                                perf_mode=DR)
                    fetch_gates(gc + 2)
                    # sigmoid(x) = 0.5*(1 + tanh(x/2)); the affine folds
                    # into the Exp per-partition scale/bias and the xb mult
                    trg = t16.tile([128, TC], BF16, tag="t16", name="trg")
                    nc.scalar.activation(trg[:], prg[:], AF.Tanh,
                                         bias=cc_col(gc, 6),
                                         scale=0.5 / WRG_SCALE)
                    sig = t16.tile([128, TC], BF16, tag="t16", name="sig")
                    nc.scalar.activation(sig[:], pig[:], AF.Tanh,
                                         bias=cc_col(gc, 5), scale=0.5)
                    # alpha = exp(negr2*(t+1)); alpha^2 = exp(negr*(t+1))
                    al = alp.tile([128, TC], F32, tag="al", name="al")
                    nc.scalar.activation(al[:], trg[:], AF.Exp,
                                         bias=cc_col(gc, 7),
                                         scale=cc_col(gc, 7))
                    a2 = a2p.tile([128, TC], F32, tag="a2", name="a2")
                    nc.vector.tensor_tensor(a2[:], al[:], al[:], OP.mult)
                    A2_t[gc] = a2
                    AL_t[gc] = al
                    SI_t[gc] = sig
                    if debug and gc == 0:
                        for idx, tl in ((0, prg), (1, pig)):
                            dt_ = alp.tile([128, TC], F32, tag="al",
                                            name="dt_")
                            nc.scalar.copy(dt_[:, 0:TC], tl[:])
                            nc.sync.dma_start(dbg_g0[idx], dt_[:, 0:TC])
                        for idx, tl in ((2, trg), (3, sig)):
                            dt_ = alp.tile([128, TC], F32, tag="al",
                                            name="dt_")
                            nc.vector.tensor_copy(dt_[:, 0:TC], tl[:])
                            nc.sync.dma_start(dbg_g0[idx], dt_[:, 0:TC])
                        nc.sync.dma_start(dbg_g0[4], al[:])
                        nc.sync.dma_start(dbg_g0[5], a2[:])

            def emit_2b(gcs, A2_t, AL_t, SI_t):
                BH_t = {}
                for gc in gcs:
                    # beta/2 = sqrt(0.25*1.000001 - 0.25*alpha^2), affine
                    # folded into the Sqrt activation read
                    bh = bhp.tile([128, TC], BF16, tag="bh", name="bh")
                    nc.scalar.activation(bh[:], A2_t[gc][:],
                                         AF.Sqrt,
                                         bias=ct[:, NCC * NH + 3:
                                                 NCC * NH + 4],
                                         scale=-0.25)
                    BH_t[gc] = bh
                for gc in gcs:
                    al, sig, bh = AL_t[gc], SI_t[gc], BH_t[gc]
                    # xbeta = (sig+1) * u * beta/2
                    xb = xbp.tile([128, TC], BF16, tag="xb", name="xb")
                    nc.vector.scalar_tensor_tensor(
                        xb[:], sig[:], 1.0, ucb_t[gc][:],
                        OP.add, OP.mult)
                    nc.vector.tensor_tensor(xb[:], xb[:],
                                            bh[:], OP.mult)
                    # scans (fp32 state feedback; bf16 out = single rounding)
                    hl = hlp.tile([128, TC], BF16, tag="hl", name="hl")
                    nc.vector.tensor_tensor_scan(
                        hl[:], al[:], xb[:], 0.0,
                        OP.mult, OP.add)
                    pp = ptp.tile([128, TC], BF16, tag="pp", name="pp")
                    nc.vector.tensor_tensor_scan(
                        pp[:], al[:], al[:], 1.0,
                        OP.mult, OP.bypass)
                    ca = gc if gc < 6 else 6 + gc
                    cb = 6 + gc if gc < 6 else 12 + gc
                    nc.vector.tensor_copy(S_loc[:, ca:ca + 1],
                                          pp[:, TC - 1:TC])
                    nc.vector.tensor_copy(S_loc[:, cb:cb + 1],
                                          hl[:, TC - 1:TC])
                    hl_t[gc] = hl
                    pt_t[gc] = pp
                    if debug and gc == 0:
                        nc.sync.dma_start(dbg_sc[0], bh[:])
                        nc.sync.dma_start(dbg_sc[1], xb[:])
                        for idx, tl in ((2, hl), (3, pp)):
                            dt_ = alp.tile([128, TC], F32, tag="al",
                                            name="dt_")
                            nc.vector.tensor_copy(dt_[:, 0:TC], tl[:])
                            nc.sync.dma_start(dbg_sc[idx], dt_[:, 0:TC])

            wt25: dict = {}

            def fetch_proj25(i):
                if i >= NH or i in wt25:
                    return
                wh = wp.tile([128, 4, 2, 128], FP8, tag="wh", name="wh")
                nc.sync.dma_start(wh[:], wp8h[NH + i])
                wl = wp.tile([128, 4, 2, 128], FP8, tag="wl", name="wl")
                nc.sync.dma_start(wl[:], wp8l[NH + i])
                wt25[i] = (wh, wl)

            G_h = [None, None]

            def emit_gather_half(h):
                # gather (A, b) summaries for hidden blocks h*6..h*6+5 within
                # the batch group, then compose this half's carry
                cin = dram.tile([128, 12], F32, tag=f"cin{h}", name=f"cin{h}")
                cout = dram.tile([4, 128, 12], F32, tag=f"cout{h}",
                                 name=f"cout{h}")
                nc.scalar.dma_start(cin[:], S_loc[:, h * 12:(h + 1) * 12])
                nc.gpsimd.collective_compute(
                    "AllGather", OP.bypass,
                    replica_groups=[[0, 1, 2, 3], [4, 5, 6, 7]],
                    ins=[cin.opt()], outs=[cout.opt()])
                G = cst.tile([128, 48], F32, tag=f"g{h}", name=f"g{h}")
                for r in range(4):
                    nc.sync.dma_start(G[:, r * 12:(r + 1) * 12], cout[r])
                G_h[h] = G
                # carry: p1 = b_0 ; p_r+1 = A_r*p_r + b_r ; c = sum sel_r*p_r
                sel = ct[:, NCC * NH:NCC * NH + 3]
                cs = slice(h * 6, (h + 1) * 6)
                p1 = G[:, 6:12]
                p2 = p2_t[:, 0:6]
                p3 = p3_t[:, 0:6]
                nc.vector.tensor_tensor(p2, G[:, 12:18], p1, OP.mult)
                nc.vector.tensor_tensor(p2, p2, G[:, 18:24], OP.add)
                nc.vector.tensor_tensor(p3, G[:, 24:30], p2, OP.mult)
                nc.vector.tensor_tensor(p3, p3, G[:, 30:36], OP.add)
                nc.vector.tensor_scalar(c_t[:, cs], p1, sel[:, 0:1], None,
                                        OP.mult)
                nc.vector.scalar_tensor_tensor(c_t[:, cs], p2, sel[:, 1:2],
                                               c_t[:, cs], OP.mult, OP.add)
                nc.vector.scalar_tensor_tensor(c_t[:, cs], p3, sel[:, 2:3],
                                               c_t[:, cs], OP.mult, OP.add)

            sizes = [3, 3, 3, 2, 1]
            starts = [sum(sizes[:i]) for i in range(len(sizes))]
            for bi, (s0, sz) in enumerate(zip(starts, sizes)):
                gcs = range(s0, s0 + sz)
                A2_t: dict = {}
                AL_t: dict = {}
                SI_t: dict = {}
                emit_2a(gcs, A2_t, AL_t, SI_t)
                if bi == 1:
                    fetch_proj25(0)
                if bi == len(sizes) - 2:
                    fetch_proj25(1)
                emit_2b(gcs, A2_t, AL_t, SI_t)
                if bi == 1:
                    emit_gather_half(0)

            # ---- phase 2.5: gate half of proj + gelu (hides the gather);
            # also fold gelu(gate) into h_loc and P in place ----
            for i in range(NH):
                fetch_proj25(i)
                pt = ps.tile([128, TC], F32, tag="ps", name="pt")
                proj_matmuls(pt, None, *wt25[i])
                fetch_proj25(i + 1)
                gg = t16.tile([128, TC], BF16, tag="t16", name="gg")
                nc.scalar.activation(gg[:], pt[:], AF.Gelu,
                                     scale=1.0 / 32.0)
                nc.vector.tensor_tensor(hl_t[i][:], gg[:], hl_t[i][:],
                                        OP.mult)
                nc.vector.tensor_tensor(pt_t[i][:], gg[:], pt_t[i][:],
                                        OP.mult)

            emit_gather_half(1)

            wo = {}

            def fetch_wout(dc):
                if dc >= NDC or dc in wo:
                    return
                w8 = wop.tile([128, NH * 128], BF16, tag="wo", name="w8")
                nc.sync.dma_start(w8[:], wout[dc])
                wo[dc] = w8

            fetch_wout(0)
            fetch_wout(1)
            if debug:
                nc.sync.dma_start(dbg_sm[0], S_loc[:])
                dt_ = cst.tile([128, 24], F32, tag="dbg24", name="dt24")
                nc.vector.tensor_copy(dt_[:, 0:12], c_t[:])
                nc.vector.tensor_copy(dt_[:, 12:24], c_t[:, 0:12])
                nc.sync.dma_start(dbg_sm[1], dt_[:])
                dt2_ = cst.tile([128, 24], F32, tag="dbg25", name="dt25")
                nc.vector.tensor_copy(dt2_[:, 0:24], S_loc[:, 0:24])
                nc.sync.dma_start(dbg_sm[2], dt2_[:])

            # ---- phase 3: h correction + output projection ----
            gh_t = []
            for gc in range(NH):
                gh = ucg.tile([128, TC], BF16, tag="ucg", name="gh")
                nc.vector.scalar_tensor_tensor(
                    gh[:], pt_t[gc][:], c_t[:, gc:gc + 1], hl_t[gc][:],
                    OP.mult, OP.add)
                gh_t.append(gh)

            for dc in range(NDC):
                po = ps.tile([128, TC], F32, tag="ps", name=f"po{dc}")
                for gc in range(NH):
                    for th in range(2):
                        nc.tensor.matmul(
                            po[:, th * 512:(th + 1) * 512],
                            wo[dc][:, gc * 128:(gc + 1) * 128],
                            gh_t[gc][:, th * 512:(th + 1) * 512],
                            start=(gc == 0), stop=(gc == NH - 1))
                fetch_wout(dc + 2)
                ot = t16.tile([128, TC], BF16, tag="t16", name="ot")
                if dc == NDC - 1:
                    ot2 = t16.tile([128, TC], BF16, tag="t16", name="ot2")
                    nc.scalar.copy(ot[:, 0:512], po[:, 0:512])
                    nc.sync.dma_start(out[dc][:, 0:512], ot[:, 0:512])
                    nc.vector.tensor_copy(ot2[:, 512:1024], po[:, 512:1024])
                    nc.scalar.dma_start(out[dc][:, 512:1024],
                                        ot2[:, 512:1024])
                elif dc % 2 == 0:
                    nc.scalar.copy(ot[:], po[:])
                    nc.sync.dma_start(out[dc], ot[:])
                else:
                    nc.vector.tensor_copy(ot[:], po[:])
                    nc.sync.dma_start(out[dc], ot[:])

    nc.compile()
    return nc


def _softplus64(x):
    x = np.asarray(x, np.float64)
    return np.log1p(np.exp(-np.abs(x))) + np.maximum(x, 0.0)


def _prepare(x, W_proj, conv_w, conv_b, W_in, b_in, W_gate, b_gate,
             forget_lambda, W_out):
    x = np.asarray(x, np.float32)
    W_proj = np.asarray(W_proj, np.float32)
    conv_w = np.asarray(conv_w, np.float32)
    conv_b = np.asarray(conv_b, np.float32)
    W_in = np.asarray(W_in, np.float32)
    b_in = np.asarray(b_in, np.float32)
    W_gate = np.asarray(W_gate, np.float32)
    b_gate = np.asarray(b_gate, np.float32)
    forget_lambda = np.asarray(forget_lambda, np.float32)
    W_out = np.asarray(W_out, np.float32)

    # wproj split fp8 hi+lo, scale 32, DoubleRow-paired over cc blocks:
    # wp8[mc][k, p, i, m] = fp8(32*W[mc*128+m, (2p+i)*128+k]).
    # mc 0..11 = u rows (1536:3072), mc 12..23 = gate rows (0:1536)
    wp_ = W_proj.reshape(24, 128, 4, 2, 128)         # [mc, m, p, i, k]
    order = list(range(12, 24)) + list(range(0, 12))
    wt_ = 32.0 * wp_[order].transpose(0, 4, 2, 3, 1)  # [mc, k, p, i, m]
    wp8h = np.ascontiguousarray(wt_).astype(NPFP8)
    wp8l = np.ascontiguousarray(
        wt_ - wp8h.astype(np.float32)).astype(NPFP8)

    wi_ = W_in.reshape(NH, 128, NH, 128)             # [gc, m, hc, k]
    wig = np.ascontiguousarray(
        wi_.transpose(0, 3, 2, 1).reshape(NH, 128, NH * 128)).astype(NPBF16)

    wg_ = W_gate.reshape(NH, 128, NP2, 2, 128)       # [gc, m, p, i, k]
    wrg = np.ascontiguousarray(
        (WRG_SCALE * wg_.transpose(0, 4, 2, 3, 1))).astype(NPFP8)

    wo_ = W_out.reshape(NDC, 128, NH, 128)           # [dc, m, gc, k]
    wout = np.ascontiguousarray(
        wo_.transpose(0, 3, 2, 1).reshape(NDC, 128, NH * 128)).astype(NPBF16)

    # consts [128, NCC*NH + 4]
    negr = (-C_CONST * _softplus64(forget_lambda)).astype(np.float64)
    cols = np.zeros((NH, 128, NCC), np.float32)
    cols[:, :, 0:4] = conv_w[:, 0, :].reshape(NH, 128, KCONV)
    cols[:, :, 4] = conv_b.reshape(NH, 128)
    cols[:, :, 5] = 0.5 * b_in.reshape(NH, 128)
    cols[:, :, 6] = 0.5 * b_gate.reshape(NH, 128)
    cols[:, :, 7] = 0.5 * negr.reshape(NH, 128)
    cols[:, :, 8] = negr.reshape(NH, 128)
    cols[:, :, 9] = (negr.reshape(NH, 128) - np.log(4.0))
    cols[:, :, 10] = cols[:, :, 3] / 32.0

    in_maps = []
    for c in range(NCORE):
        bb, j = divmod(c, 4)
        full = np.zeros((TPAD, DIM), np.float32)
        if j > 0:
            full[1:4] = x[bb, j * TC - 3:j * TC]
        full[4:] = x[bb, j * TC:(j + 1) * TC]
        xb_ = np.ascontiguousarray(full[4:].T)       # [DIM, TC]
        xh = xb_.astype(NPFP8)
        xl = (xb_ - xh.astype(np.float32)).astype(NPFP8)
        x8h = np.ascontiguousarray(
            xh.reshape(4, 2, 128, 4, 256).transpose(0, 2, 3, 1, 4))
        x8l = np.ascontiguousarray(
            xl.reshape(4, 2, 128, 4, 256).transpose(0, 2, 3, 1, 4))
        xpre_ = np.ascontiguousarray(
            full[0:4].T.astype(NPFP8).reshape(4, 2, 128, 4)
            .transpose(2, 0, 1, 3))
        consts = np.zeros((128, NCC * NH + 4), np.float32)
        consts[:, NCC * NH + 3] = 0.25000025
        consts[:, :NCC * NH] = cols.transpose(1, 0, 2).reshape(
            128, NH * NCC)
        if j > 0:
            consts[:, NCC * NH + j - 1] = 1.0
        in_maps.append({
            "xt8h": x8h, "xt8l": x8l, "xpre": xpre_,
            "wp8h": wp8h, "wp8l": wp8l, "wig": wig,
            "wrg": wrg, "wout": wout, "consts": consts,
        })
    return in_maps


def _get_nc():
    if "nc" not in _CACHE:
        _CACHE["nc"] = _build()
    return _CACHE["nc"]


def kernel(x, W_proj, conv_w, conv_b, W_in, b_in, W_gate, b_gate,
           forget_lambda, W_out):
    nc = _get_nc()
    in_maps = _prepare(x, W_proj, conv_w, conv_b, W_in, b_in, W_gate, b_gate,
                       forget_lambda, W_out)
    res = run_bass_kernel_spmd(nc, in_maps, core_ids=list(range(NCORE)))
    out = np.empty((B, T, DIM), np.float32)
    for c in range(NCORE):
        bb, j = divmod(c, 4)
        o = np.asarray(res.results[c]["out"]).reshape(DIM, TC)
        out[bb, j * TC:(j + 1) * TC, :] = o.T.astype(np.float32)
    return out


# revision 53
# speedup vs baseline: 1.0021x; 1.0021x over previous
"""Hawk (RG-LRU recurrent block) Trainium2 kernel, 8-core SPMD.

Sharding: data-parallel over B (2 groups of 4 cores) x sequence-parallel over
T (4 chunks of 1024 tokens per batch element). The diagonal linear recurrence
h_t = a_t * h_{t-1} + b_t is computed chunk-locally with the hardware
tensor_tensor_scan instruction (fp32 state feedback), then stitched across
cores with one small AllGather of per-chunk scan summaries (A = prod a,
b = local final h) and the per-core correction h = h_local + cumprod(a)*carry.

Layout on device: hidden channels on partitions, time on the free dimension.

Optimizations over the f32r baseline (412us -> ~214us in the cost model):
  - input and output projections run as fp8 e4m3 DoubleRow matmuls (2 K-blocks
    per instruction at 0.5 cycles/row = 4x f32r throughput) using a hi+lo
    split: W@x ~= Wh@xh + Wl@xh + Wh@xl with both operands split host-side
    (weights scaled by 32 into fp8 range; descale folded into the per-
    partition ACT scale at the PSUM read). Residual-of-residual error is
    ~0.1%, BETTER than plain bf16.
  - the rec-gate matmul is a single (uncorrected) fp8 DoubleRow pass: alpha
    error tolerance scales with the per-channel forget rate, which is tiny
    exactly for the long-memory channels. The input-gate matmul stays bf16
    (its split-3 variant shifts the bottleneck to the vector engine).
  - sigmoid via tanh identity keeps phase 2 in one ACT table set (tanh+exp);
    sqrt batches amortize the only other table load. ~22 table loads -> ~10.
  - weights are stored pre-transposed/pre-quantized in DRAM so every load is
    a contiguous [128, N] DMA (~128 descriptors); streams are spread across
    the SP and ACT HWDGE queues with explicit prefetch depth so no in-order
    queue blocks a later phase's loads. The baseline serialized ~400us of
    descriptor generation on the SP sequencer.
  - conv runs on DVE with the unshifted tap folded into the PSUM->SBUF copy
    (ACT Identity with per-partition scale+bias); bf16 operands throughout.
  - P = cumprod(alpha) stays in SBUF (bf16, single rounding per element);
    gelu(gate) is folded into h_loc and P in place during the collective so
    the post-carry critical path is one fused STT per channel block.
  - phase 2.5 (gate projection) fully hides the 15us-fixed-cost collective;
    the carry composition and gh production overlap its tail.
"""
import contextlib
import os

os.environ.setdefault("JAX_COMPILATION_CACHE_DIR", "/tmp/jax_cache_hawk")

import numpy as np

import concourse.bacc as bacc
import concourse.mybir as mybir
import concourse.tile as tile
from concourse.bass_utils import run_bass_kernel_spmd

F32 = mybir.dt.float32
BF16 = mybir.dt.bfloat16
FP8 = mybir.dt.float8e4
AF = mybir.ActivationFunctionType
OP = mybir.AluOpType
DR = mybir.MatmulPerfMode.DoubleRow

NPBF16 = mybir.dt.np(BF16)
NPFP8 = mybir.dt.np(FP8)

DIM = 1024
HID = 1536
KCONV = 4
B = 2
T = 4096
C_CONST = 8.0

NCORE = 8
TC = T // 4          # tokens per core
NH = HID // 128      # 12 hidden chunks
NDC = DIM // 128     # 8 dim chunks
NP2 = NH // 2        # 6 hidden chunk pairs (fp8 DoubleRow)
TPAD = TC + 4        # 1028: col0 zero, cols1:4 = 3-token left overlap
NCC = 11             # const columns per hidden chunk
WRG_SCALE = 64.0     # fp8 scale for W_gate

_CACHE: dict = {}


def _build(debug=False):
    nc = bacc.Bacc("TRN2", target_bir_lowering=False, debug=False,
                   num_devices=NCORE, dynamic_dma_scratch_size=8192)

    xt8h = nc.dram_tensor("xt8h", [4, 128, 2, 1024], FP8,
                          kind="ExternalInput").ap()
    xt8l = nc.dram_tensor("xt8l", [4, 128, 2, 1024], FP8,
                          kind="ExternalInput").ap()
    xpre = nc.dram_tensor("xpre", [128, 4, 2, 4], FP8,
                          kind="ExternalInput").ap()
    wp8h = nc.dram_tensor("wp8h", [24, 128, 4, 2, 128], FP8,
                          kind="ExternalInput").ap()
    wp8l = nc.dram_tensor("wp8l", [24, 128, 4, 2, 128], FP8,
                          kind="ExternalInput").ap()
    wig = nc.dram_tensor("wig", [NH, 128, NH * 128], BF16,
                         kind="ExternalInput").ap()
    wrg = nc.dram_tensor("wrg", [NH, 128, NP2, 2, 128], FP8,
                         kind="ExternalInput").ap()
    wout = nc.dram_tensor("wout", [NDC, 128, NH * 128], BF16,
                          kind="ExternalInput").ap()
    consts = nc.dram_tensor("consts", [128, NCC * NH + 4], F32,
                            kind="ExternalInput").ap()
    out = nc.dram_tensor("out", [NDC, 128, TC], BF16, kind="ExternalOutput").ap()
    if debug:
        dbg_ucb = nc.dram_tensor("dbg_ucb", [NH, 128, TC], F32,
                                 kind="ExternalOutput").ap()
        dbg_g0 = nc.dram_tensor("dbg_g0", [6, 128, TC], F32,
                                kind="ExternalOutput").ap()
        dbg_sc = nc.dram_tensor("dbg_sc", [4, 128, TC], F32,
                                kind="ExternalOutput").ap()
        dbg_sm = nc.dram_tensor("dbg_sm", [3, 128, 24], F32,
                                kind="ExternalOutput").ap()

    with tile.TileContext(nc) as tc:
        with contextlib.ExitStack() as stk:
            def pool(name, bufs, space="SBUF"):
                return stk.enter_context(
                    tc.tile_pool(name=name, bufs=bufs, space=space))

            xtp = pool("xtp", NDC)
            ucg = pool("ucg", NH)          # ucb then gh
            u8p = pool("u8p", NP2)
            hlp = pool("hlp", NH)
            ptp = pool("ptp", NH)
            t16 = pool("t16", 6)
            b16p = pool("b16p", 6)         # upre/tmp (conv)
            alp = pool("alp", 5)
            a2p = pool("a2p", 5)
            bhp = pool("bhp", 6)
            xbp = pool("xbp", 2)
            wp = pool("wp", 5)
            wrgp = pool("wrgp", 3)
            wigp = pool("wigp", 3)
            wop = pool("wop", 3)
            cst = pool("cst", 1)
            ps = pool("ps", 3, space="PSUM")
            psb = pool("psb", 1, space="PSUM")
            dram = pool("dram", 1, space="DRAM")

            # ---- x^T + first weights (PE work is gated on these) ----
            # fp8 hi+lo split, DoubleRow-paired over input-channel blocks
            wpr_t: dict = {}

            def fetch_proj1(mc):
                if mc >= NH or mc in wpr_t:
                    return
                wh = wp.tile([128, 4, 2, 128], FP8, tag="wh", name="wh")
                nc.sync.dma_start(wh[:], wp8h[mc])
                wl = wp.tile([128, 4, 2, 128], FP8, tag="wl", name="wl")
                nc.sync.dma_start(wl[:], wp8l[mc])
                wpr_t[mc] = (wh, wl)

            wh0 = wp.tile([128, 4, 2, 128], FP8, tag="wh", name="wh")
            nc.sync.dma_start(wh0[:], wp8h[0])
            x8h_t = []
            x8l_t = []
            for p in range(4):
                xh = xtp.tile([128, 2, 1024], FP8, tag="xt", name="xh")
                nc.sync.dma_start(xh[:], xt8h[p])
                x8h_t.append(xh)
            wl0 = wp.tile([128, 4, 2, 128], FP8, tag="wl", name="wl")
            nc.sync.dma_start(wl0[:], wp8l[0])
            wpr_t[0] = (wh0, wl0)
            for p in range(4):
                xl = xtp.tile([128, 2, 1024], FP8, tag="xt", name="xl")
                nc.sync.dma_start(xl[:], xt8l[p])
                x8l_t.append(xl)
            fetch_proj1(1)

            # ---- constants: one DMA (ACT queue; conv reads them there) ----
            ct = cst.tile([128, NCC * NH + 4], F32, tag="ct", name="ct")
            nc.scalar.dma_start(ct[:], consts[:])

            def cc_col(gc, c):
                return ct[:, gc * NCC + c: gc * NCC + c + 1]

            S_loc = cst.tile([128, 24], F32, tag="sloc", name="S_loc")
            p2_t = cst.tile([128, NH], F32, tag="p2", name="p2_t")
            p3_t = cst.tile([128, NH], F32, tag="p3", name="p3_t")
            c_t = cst.tile([128, NH], F32, tag="cc", name="c_t")

            def proj_matmuls(pt, p3t, wh, wl):
                # W@x = Wh@xh + Wh@xl + Wl@xh (lo*lo dropped), all fp8
                # DoubleRow at 4x f32r rate; scale 32 on W folded out at the
                # PSUM read.
                passes = ((wh, x8h_t), (wl, x8h_t), (wh, x8l_t))
                for q in range(2):
                    for n, (w_, x_) in enumerate(passes):
                        for p in range(4):
                            nc.tensor.matmul(
                                pt[:, q * 512:(q + 1) * 512],
                                w_[:, p],
                                x_[p][:, :, q * 512:(q + 1) * 512],
                                start=(n == 0 and p == 0),
                                stop=(n == 2 and p == 3),
                                perf_mode=DR)
                if p3t is not None:
                    for p in range(4):
                        nc.tensor.matmul(
                            p3t[:, 0:4],
                            wh[:, p],
                            xpre_t[:, p],
                            start=(p == 0), stop=(p == 3),
                            perf_mode=DR)

            # ---- phase 1: u half of proj + causal conv ----
            ucb_t = []
            u8_t = []
            for p in range(NP2):
                u8 = u8p.tile([128, 2, 1024], FP8, tag="u8", name="u8")
                u8_t.append(u8)
            xpre_t = cst.tile([128, 4, 2, 4], FP8, tag="xpre", name="xpre_t")
            nc.sync.dma_start(xpre_t[:], xpre[:])
            for mc in range(NH):
                fetch_proj1(mc)
                wh, wl = wpr_t[mc]
                pt = ps.tile([128, TC], F32, tag="ps", name="pt")
                p3t = psb.tile([128, TC], F32, tag="psb", name="p3t")
                proj_matmuls(pt, p3t, wh, wl)
                fetch_proj1(mc + 2)
                upre = b16p.tile([128, TPAD], BF16, tag="b16", name="upre")
                tmp = b16p.tile([128, TPAD], BF16, tag="b16", name="tmp")
                ucb = ucg.tile([128, TC], BF16, tag="ucg", name="ucb")
                # u_c[t] = sum_k w_k * u_pre[t+k+1] + conv_b; the k=3 tap is
                # unshifted, so it folds into the PSUM->SBUF read via ACT
                # scale/bias (tmp = w_3 * u_pre_body + conv_b); the last two
                # blocks run at half-tile granularity so the phase-2 gate
                # matmuls are not gated on a full-width conv chain.
                halves = (0, 1) if mc >= NH - 2 else (None,)
                for hh in halves:
                    if hh is None:
                        ps_sl = slice(0, TC)
                    else:
                        ps_sl = slice(hh * 512, (hh + 1) * 512)
                    up_sl = slice(4 + ps_sl.start, 4 + ps_sl.stop)
                    nc.scalar.activation(upre[:, up_sl], pt[:, ps_sl],
                                         AF.Identity, scale=1.0 / 32.0)
                    nc.scalar.activation(tmp[:, ps_sl], pt[:, ps_sl],
                                         AF.Identity, bias=cc_col(mc, 4),
                                         scale=cc_col(mc, 10))
                nc.vector.tensor_scalar(upre[:, 0:4], p3t[:, 0:4],
                                        1.0 / 32.0, None, OP.mult)
                for hh in halves:
                    t_sl = (slice(0, TC) if hh is None
                            else slice(hh * 512, (hh + 1) * 512))
                    n_ = t_sl.stop - t_sl.start
                    for k in (0, 1):
                        nc.vector.scalar_tensor_tensor(
                            tmp[:, t_sl],
                            upre[:, 1 + k + t_sl.start:1 + k + t_sl.start + n_],
                            cc_col(mc, k), tmp[:, t_sl], OP.mult, OP.add)
                    nc.vector.scalar_tensor_tensor(
                        ucb[:, t_sl],
                        upre[:, 3 + t_sl.start:3 + t_sl.start + n_],
                        cc_col(mc, 2), tmp[:, t_sl], OP.mult, OP.add)
                ucb_t.append(ucb)
                # fp8 copy for the rec-gate DoubleRow matmul, th-interleaved
                p, i = divmod(mc, 2)
                nc.scalar.copy(u8_t[p][:, i], ucb[:])


            if debug:
                for mc in range(NH):
                    dt_ = alp.tile([128, TC], F32, tag="al", name="dt_")
                    nc.vector.tensor_copy(dt_[:, 0:TC], ucb_t[mc][:])
                    nc.sync.dma_start(dbg_ucb[mc], dt_[:, 0:TC])

            # ---- phase 2: gates + scans, in ACT-table batches ----
            # tanh/exp share one ACT table set; Sqrt lives in another, so
            # the per-batch Sqrt ops are grouped to amortize table loads.
            hl_t: list = [None] * NH
            pt_t: list = [None] * NH

            wi_t: dict = {}
            wr_t: dict = {}

            def fetch_gates(gc):
                if gc >= NH:
                    return
                wi = wigp.tile([128, NH * 128], BF16, tag="wi", name="wi")
                nc.sync.dma_start(wi[:], wig[gc])
                wi_t[gc] = wi
                wr = wrgp.tile([128, NP2, 2, 128], FP8, tag="wr", name="wr")
                nc.sync.dma_start(wr[:], wrg[gc])
                wr_t[gc] = wr

            fetch_gates(0)
            fetch_gates(1)

            def emit_2a(gcs, A2_t, AL_t, SI_t):
                for gc in gcs:
                    # input gate: bf16 (needs only ucb -> starts earliest)
                    pig = ps.tile([128, TC], F32, tag="ps", name="pig")
                    for th in range(2):
                        for hc in range(NH):
                            nc.tensor.matmul(
                                pig[:, th * 512:(th + 1) * 512],
                                wi_t[gc][:, hc * 128:(hc + 1) * 128],
                                ucb_t[hc][:, th * 512:(th + 1) * 512],
                                start=(hc == 0), stop=(hc == NH - 1))
                    # rec gate: fp8 e4m3 DoubleRow (2 K-chunks per matmul)
                    prg = psb.tile([128, TC], F32, tag="psb", name="prg")
                    for q in range(2):
                        for p in range(NP2):
                            nc.tensor.matmul(
                                prg[:, q * 512:(q + 1) * 512],
                                wr_t[gc][:, p],
                                u8_t[p][:, :, q * 512:(q + 1) * 512],
                                start=(p == 0), stop=(p == NP2 - 1),
                                perf_mode=DR)
                    fetch_gates(gc + 2)
                    # sigmoid(x) = 0.5*(1 + tanh(x/2)); the affine folds
                    # into the Exp per-partition scale/bias and the xb mult
                    trg = t16.tile([128, TC], BF16, tag="t16", name="trg")
                    nc.scalar.activation(trg[:], prg[:], AF.Tanh,
                                         bias=cc_col(gc, 6),
                                         scale=0.5 / WRG_SCALE)
                    sig = t16.tile([128, TC], BF16, tag="t16", name="sig")
                    nc.scalar.activation(sig[:], pig[:], AF.Tanh,
                                         bias=cc_col(gc, 5), scale=0.5)
                    # alpha = exp(negr2*(t+1)); alpha^2 = exp(negr*(t+1))
                    al = alp.tile([128, TC], F32, tag="al", name="al")
                    nc.scalar.activation(al[:], trg[:], AF.Exp,
                                         bias=cc_col(gc, 7),
                                         scale=cc_col(gc, 7))
                    a2 = a2p.tile([128, TC], F32, tag="a2", name="a2")
                    nc.vector.tensor_tensor(a2[:], al[:], al[:], OP.mult)
                    A2_t[gc] = a2
                    AL_t[gc] = al
                    SI_t[gc] = sig
                    if debug and gc == 0:
                        for idx, tl in ((0, prg), (1, pig)):
                            dt_ = alp.tile([128, TC], F32, tag="al",
                                            name="dt_")
                            nc.scalar.copy(dt_[:, 0:TC], tl[:])
                            nc.sync.dma_start(dbg_g0[idx], dt_[:, 0:TC])
                        for idx, tl in ((2, trg), (3, sig)):
                            dt_ = alp.tile([128, TC], F32, tag="al",
                                            name="dt_")
                            nc.vector.tensor_copy(dt_[:, 0:TC], tl[:])
                            nc.sync.dma_start(dbg_g0[idx], dt_[:, 0:TC])
                        nc.sync.dma_start(dbg_g0[4], al[:])
                        nc.sync.dma_start(dbg_g0[5], a2[:])

            def emit_2b(gcs, A2_t, AL_t, SI_t):
                BH_t = {}
                for gc in gcs:
                    # beta/2 = sqrt(0.25*1.000001 - 0.25*alpha^2), affine
                    # folded into the Sqrt activation read
                    bh = bhp.tile([128, TC], BF16, tag="bh", name="bh")
                    nc.scalar.activation(bh[:], A2_t[gc][:],
                                         AF.Sqrt,
                                         bias=ct[:, NCC * NH + 3:
                                                 NCC * NH + 4],
                                         scale=-0.25)
                    BH_t[gc] = bh
                for gc in gcs:
                    al, sig, bh = AL_t[gc], SI_t[gc], BH_t[gc]
                    # xbeta = (sig+1) * u * beta/2
                    xb = xbp.tile([128, TC], BF16, tag="xb", name="xb")
                    nc.vector.scalar_tensor_tensor(
                        xb[:], sig[:], 1.0, ucb_t[gc][:],
                        OP.add, OP.mult)
                    nc.vector.tensor_tensor(xb[:], xb[:],
                                            bh[:], OP.mult)
                    # scans (fp32 state feedback; bf16 out = single rounding)
                    hl = hlp.tile([128, TC], BF16, tag="hl", name="hl")
                    nc.vector.tensor_tensor_scan(
                        hl[:], al[:], xb[:], 0.0,
                        OP.mult, OP.add)
                    pp = ptp.tile([128, TC], BF16, tag="pp", name="pp")
                    nc.vector.tensor_tensor_scan(
                        pp[:], al[:], al[:], 1.0,
                        OP.mult, OP.bypass)
                    ca = gc if gc < 6 else 6 + gc
                    cb = 6 + gc if gc < 6 else 12 + gc
                    nc.vector.tensor_copy(S_loc[:, ca:ca + 1],
                                          pp[:, TC - 1:TC])
                    nc.vector.tensor_copy(S_loc[:, cb:cb + 1],
                                          hl[:, TC - 1:TC])
                    hl_t[gc] = hl
                    pt_t[gc] = pp
                    if debug and gc == 0:
                        nc.sync.dma_start(dbg_sc[0], bh[:])
                        nc.sync.dma_start(dbg_sc[1], xb[:])
                        for idx, tl in ((2, hl), (3, pp)):
                            dt_ = alp.tile([128, TC], F32, tag="al",
                                            name="dt_")
                            nc.vector.tensor_copy(dt_[:, 0:TC], tl[:])
                            nc.sync.dma_start(dbg_sc[idx], dt_[:, 0:TC])

            wt25: dict = {}

            def fetch_proj25(i):
                if i >= NH or i in wt25:
                    return
                wh = wp.tile([128, 4, 2, 128], FP8, tag="wh", name="wh")
                nc.sync.dma_start(wh[:], wp8h[NH + i])
                wl = wp.tile([128, 4, 2, 128], FP8, tag="wl", name="wl")
                nc.sync.dma_start(wl[:], wp8l[NH + i])
                wt25[i] = (wh, wl)

            G_h = [None, None]

            def emit_gather_half(h):
                # gather (A, b) summaries for hidden blocks h*6..h*6+5 within
                # the batch group, then compose this half's carry
                cin = dram.tile([128, 12], F32, tag=f"cin{h}", name=f"cin{h}")
                cout = dram.tile([4, 128, 12], F32, tag=f"cout{h}",
                                 name=f"cout{h}")
                nc.scalar.dma_start(cin[:], S_loc[:, h * 12:(h + 1) * 12])
                nc.gpsimd.collective_compute(
                    "AllGather", OP.bypass,
                    replica_groups=[[0, 1, 2, 3], [4, 5, 6, 7]],
                    ins=[cin.opt()], outs=[cout.opt()])
                G = cst.tile([128, 48], F32, tag=f"g{h}", name=f"g{h}")
                for r in range(4):
                    nc.sync.dma_start(G[:, r * 12:(r + 1) * 12], cout[r])
                G_h[h] = G
                # carry: p1 = b_0 ; p_r+1 = A_r*p_r + b_r ; c = sum sel_r*p_r
                sel = ct[:, NCC * NH:NCC * NH + 3]
                cs = slice(h * 6, (h + 1) * 6)
                p1 = G[:, 6:12]
                p2 = p2_t[:, 0:6]
                p3 = p3_t[:, 0:6]
                nc.vector.tensor_tensor(p2, G[:, 12:18], p1, OP.mult)
                nc.vector.tensor_tensor(p2, p2, G[:, 18:24], OP.add)
                nc.vector.tensor_tensor(p3, G[:, 24:30], p2, OP.mult)
                nc.vector.tensor_tensor(p3, p3, G[:, 30:36], OP.add)
                nc.vector.tensor_scalar(c_t[:, cs], p1, sel[:, 0:1], None,
                                        OP.mult)
                nc.vector.scalar_tensor_tensor(c_t[:, cs], p2, sel[:, 1:2],
                                               c_t[:, cs], OP.mult, OP.add)
                nc.vector.scalar_tensor_tensor(c_t[:, cs], p3, sel[:, 2:3],
                                               c_t[:, cs], OP.mult, OP.add)

            sizes = [3, 3, 3, 2, 1]
            starts = [sum(sizes[:i]) for i in range(len(sizes))]
            for bi, (s0, sz) in enumerate(zip(starts, sizes)):
                gcs = range(s0, s0 + sz)
                A2_t: dict = {}
                AL_t: dict = {}
                SI_t: dict = {}
                emit_2a(gcs, A2_t, AL_t, SI_t)
                if bi == 1:
                    fetch_proj25(0)
                if bi == len(sizes) - 2:
                    fetch_proj25(1)
                emit_2b(gcs, A2_t, AL_t, SI_t)
                if bi == 1:
                    emit_gather_half(0)

            # ---- phase 2.5: gate half of proj + gelu (hides the gather);
            # also fold gelu(gate) into h_loc and P in place ----
            for i in range(NH):
                fetch_proj25(i)
                pt = ps.tile([128, TC], F32, tag="ps", name="pt")
                proj_matmuls(pt, None, *wt25[i])
                fetch_proj25(i + 1)
                gg = t16.tile([128, TC], BF16, tag="t16", name="gg")
                nc.scalar.activation(gg[:], pt[:], AF.Gelu,
                                     scale=1.0 / 32.0)
                nc.vector.tensor_tensor(hl_t[i][:], gg[:], hl_t[i][:],
                                        OP.mult)
                nc.vector.tensor_tensor(pt_t[i][:], gg[:], pt_t[i][:],
                                        OP.mult)

            emit_gather_half(1)

            wo = {}

            def fetch_wout(dc):
                if dc >= NDC or dc in wo:
                    return
                w8 = wop.tile([128, NH * 128], BF16, tag="wo", name="w8")
                nc.sync.dma_start(w8[:], wout[dc])
                wo[dc] = w8

            fetch_wout(0)
            fetch_wout(1)
            if debug:
                nc.sync.dma_start(dbg_sm[0], S_loc[:])
                dt_ = cst.tile([128, 24], F32, tag="dbg24", name="dt24")
                nc.vector.tensor_copy(dt_[:, 0:12], c_t[:])
                nc.vector.tensor_copy(dt_[:, 12:24], c_t[:, 0:12])
                nc.sync.dma_start(dbg_sm[1], dt_[:])
                dt2_ = cst.tile([128, 24], F32, tag="dbg25", name="dt25")
                nc.vector.tensor_copy(dt2_[:, 0:24], S_loc[:, 0:24])
                nc.sync.dma_start(dbg_sm[2], dt2_[:])

            # ---- phase 3: h correction + output projection ----
            gh_t = []
            for gc in range(NH):
                gh = ucg.tile([128, TC], BF16, tag="ucg", name="gh")
                nc.vector.scalar_tensor_tensor(
                    gh[:], pt_t[gc][:], c_t[:, gc:gc + 1], hl_t[gc][:],
                    OP.mult, OP.add)
                gh_t.append(gh)

            for dc in range(NDC):
                po = ps.tile([128, TC], F32, tag="ps", name=f"po{dc}")
                for gc in range(NH):
                    for th in range(2):
                        nc.tensor.matmul(
                            po[:, th * 512:(th + 1) * 512],
                            wo[dc][:, gc * 128:(gc + 1) * 128],
                            gh_t[gc][:, th * 512:(th + 1) * 512],
                            start=(gc == 0), stop=(gc == NH - 1))
                fetch_wout(dc + 2)
                ot = t16.tile([128, TC], BF16, tag="t16", name="ot")
                if dc == NDC - 1:
                    ot2 = t16.tile([128, TC], BF16, tag="t16", name="ot2")
                    nc.scalar.copy(ot[:, 0:512], po[:, 0:512])
                    nc.sync.dma_start(out[dc][:, 0:512], ot[:, 0:512])
                    nc.vector.tensor_copy(ot2[:, 512:1024], po[:, 512:1024])
                    nc.scalar.dma_start(out[dc][:, 512:1024],
                                        ot2[:, 512:1024])
                elif dc % 2 == 0:
                    nc.scalar.copy(ot[:], po[:])
                    nc.sync.dma_start(out[dc], ot[:])
                else:
                    nc.vector.tensor_copy(ot[:], po[:])
                    nc.sync.dma_start(out[dc], ot[:])

    nc.compile()
    return nc


def _softplus64(x):
    x = np.asarray(x, np.float64)
    return np.log1p(np.exp(-np.abs(x))) + np.maximum(x, 0.0)


def _prepare(x, W_proj, conv_w, conv_b, W_in, b_in, W_gate, b_gate,
             forget_lambda, W_out):
    x = np.asarray(x, np.float32)
    W_proj = np.asarray(W_proj, np.float32)
    conv_w = np.asarray(conv_w, np.float32)
    conv_b = np.asarray(conv_b, np.float32)
    W_in = np.asarray(W_in, np.float32)
    b_in = np.asarray(b_in, np.float32)
    W_gate = np.asarray(W_gate, np.float32)
    b_gate = np.asarray(b_gate, np.float32)
    forget_lambda = np.asarray(forget_lambda, np.float32)
    W_out = np.asarray(W_out, np.float32)

    # wproj split fp8 hi+lo, scale 32, DoubleRow-paired over cc blocks:
    # wp8[mc][k, p, i, m] = fp8(32*W[mc*128+m, (2p+i)*128+k]).
    # mc 0..11 = u rows (1536:3072), mc 12..23 = gate rows (0:1536)
    wp_ = W_proj.reshape(24, 128, 4, 2, 128)         # [mc, m, p, i, k]
    order = list(range(12, 24)) + list(range(0, 12))
    wt_ = 32.0 * wp_[order].transpose(0, 4, 2, 3, 1)  # [mc, k, p, i, m]
    wp8h = np.ascontiguousarray(wt_).astype(NPFP8)
    wp8l = np.ascontiguousarray(
        wt_ - wp8h.astype(np.float32)).astype(NPFP8)

    wi_ = W_in.reshape(NH, 128, NH, 128)             # [gc, m, hc, k]
    wig = np.ascontiguousarray(
        wi_.transpose(0, 3, 2, 1).reshape(NH, 128, NH * 128)).astype(NPBF16)

    wg_ = W_gate.reshape(NH, 128, NP2, 2, 128)       # [gc, m, p, i, k]
    wrg = np.ascontiguousarray(
        (WRG_SCALE * wg_.transpose(0, 4, 2, 3, 1))).astype(NPFP8)

    wo_ = W_out.reshape(NDC, 128, NH, 128)           # [dc, m, gc, k]
    wout = np.ascontiguousarray(
        wo_.transpose(0, 3, 2, 1).reshape(NDC, 128, NH * 128)).astype(NPBF16)

    # consts [128, NCC*NH + 4]
    negr = (-C_CONST * _softplus64(forget_lambda)).astype(np.float64)
    cols = np.zeros((NH, 128, NCC), np.float32)
    cols[:, :, 0:4] = conv_w[:, 0, :].reshape(NH, 128, KCONV)
    cols[:, :, 4] = conv_b.reshape(NH, 128)
    cols[:, :, 5] = 0.5 * b_in.reshape(NH, 128)
    cols[:, :, 6] = 0.5 * b_gate.reshape(NH, 128)
    cols[:, :, 7] = 0.5 * negr.reshape(NH, 128)
    cols[:, :, 8] = negr.reshape(NH, 128)
    cols[:, :, 9] = (negr.reshape(NH, 128) - np.log(4.0))
    cols[:, :, 10] = cols[:, :, 3] / 32.0

    in_maps = []
    for c in range(NCORE):
        bb, j = divmod(c, 4)
        full = np.zeros((TPAD, DIM), np.float32)
        if j > 0:
            full[1:4] = x[bb, j * TC - 3:j * TC]
        full[4:] = x[bb, j * TC:(j + 1) * TC]
        xb_ = np.ascontiguousarray(full[4:].T)       # [DIM, TC]
        xh = xb_.astype(NPFP8)
        xl = (xb_ - xh.astype(np.float32)).astype(NPFP8)
        x8h = np.ascontiguousarray(
            xh.reshape(4, 2, 128, 1024).transpose(0, 2, 1, 3))
        x8l = np.ascontiguousarray(
            xl.reshape(4, 2, 128, 1024).transpose(0, 2, 1, 3))
        xpre_ = np.ascontiguousarray(
            full[0:4].T.astype(NPFP8).reshape(4, 2, 128, 4)
            .transpose(2, 0, 1, 3))
        consts = np.zeros((128, NCC * NH + 4), np.float32)
        consts[:, NCC * NH + 3] = 0.25000025
        consts[:, :NCC * NH] = cols.transpose(1, 0, 2).reshape(
            128, NH * NCC)
        if j > 0:
            consts[:, NCC * NH + j - 1] = 1.0
        in_maps.append({
            "xt8h": x8h, "xt8l": x8l, "xpre": xpre_,
            "wp8h": wp8h, "wp8l": wp8l, "wig": wig,
            "wrg": wrg, "wout": wout, "consts": consts,
        })
    return in_maps


def _get_nc():
    if "nc" not in _CACHE:
        _CACHE["nc"] = _build()
    return _CACHE["nc"]


def kernel(x, W_proj, conv_w, conv_b, W_in, b_in, W_gate, b_gate,
           forget_lambda, W_out):
    nc = _get_nc()
    in_maps = _prepare(x, W_proj, conv_w, conv_b, W_in, b_in, W_gate, b_gate,
                       forget_lambda, W_out)
    res = run_bass_kernel_spmd(nc, in_maps, core_ids=list(range(NCORE)))
    out = np.empty((B, T, DIM), np.float32)
    for c in range(NCORE):
        bb, j = divmod(c, 4)
        o = np.asarray(res.results[c]["out"]).reshape(DIM, TC)
        out[bb, j * TC:(j + 1) * TC, :] = o.T.astype(np.float32)
    return out
